# revision 1
# baseline (speedup 1.0000x reference)
"""BiDAF attention kernel for 8 Trainium2 NeuronCores.

Data-parallel over batch (B=32 -> 4 per core). Per batch, on-chip:
  sT[j,i] = (q*cqw) @ c^T + s0[i] + s1[j] + bias   (bf16 matmuls, fp32 accum;
  s0 = c @ cwgt comes free as row 64 of the same matmul via an extra cwgt
  column in the lhsT, then a rank-1 augmentation matmul broadcasts it)
  s1 = q @ qw is recovered from the already-transposed qs^T via the
  host-precomputed ratio vector qw/cqw (scale-invariant in bf16).
  E = exp(sT)  (one exp serves both softmaxes; s1+bias fused via act bias)
  a1 normalization deferred: 1/colsum(E) scales downstream matmul outputs;
  a2 normalization deferred: 1/rowsum(E) folds into the M2 PSUM evacuation.
  a = a1 @ q; b = a1 @ (a2^T @ c); device stores [a, c*a, c*b] in bf16.
The c passthrough column and the bf16->f32 upcast happen on the host
during unshard (the device never touches the 4 MiB/core c column and
writes half-width outputs: 10.5 MiB HBM traffic/core instead of 20.5).
"""

import sys

if "/opt/trn_rl_repo" not in sys.path:
    sys.path.insert(0, "/opt/trn_rl_repo")

from contextlib import ExitStack

import numpy as np

import concourse.bacc as bacc
import concourse.bass as bass
import concourse.mybir as mybir
from concourse.bass import ts
from concourse.bass_utils import run_bass_kernel_spmd
from concourse.masks import make_identity
from concourse.tile import TileContext

N_CORES = 8
B, Lc, Lq, H = 32, 512, 64, 512
BPC = B // N_CORES  # batches per core
F32 = mybir.dt.float32
BF16 = mybir.dt.bfloat16
MULT = mybir.AluOpType.mult

_CACHE = {}


def _build_program():
    nc = bacc.Bacc("TRN2", target_bir_lowering=False, debug=False, num_devices=N_CORES)
    c_h = nc.dram_tensor("c", [BPC, Lc, H], F32, kind="ExternalInput")
    q_h = nc.dram_tensor("q", [BPC, Lq, H], F32, kind="ExternalInput")
    consts_h = nc.dram_tensor("consts", [3, H], F32, kind="ExternalInput")
    bias_h = nc.dram_tensor("bias", [1], F32, kind="ExternalInput")
    out_h = nc.dram_tensor("out", [BPC, Lc, 3 * H], BF16, kind="ExternalOutput")

    c_ap = c_h.ap()
    q_ap = q_h.ap()
    out_ap = out_h.ap()

    exp_f = mybir.ActivationFunctionType.Exp
    ident_f = mybir.ActivationFunctionType.Identity
    copy_f = mybir.ActivationFunctionType.Copy

    with TileContext(nc) as tc, ExitStack() as ctx:
        const = ctx.enter_context(tc.tile_pool(name="const", bufs=1))
        cpool = ctx.enter_context(tc.tile_pool(name="cpool", bufs=4))
        cbfpool = ctx.enter_context(tc.tile_pool(name="cbfpool", bufs=4))
        ctpool = ctx.enter_context(tc.tile_pool(name="ctpool", bufs=4))
        qpool = ctx.enter_context(tc.tile_pool(name="qpool", bufs=4))
        spool = ctx.enter_context(tc.tile_pool(name="spool", bufs=4))
        epool = ctx.enter_context(tc.tile_pool(name="epool", bufs=4))
        btpool = ctx.enter_context(tc.tile_pool(name="btpool", bufs=4))
        opool = ctx.enter_context(tc.tile_pool(name="opool", bufs=8))
        ps_tr = ctx.enter_context(tc.tile_pool(name="ps_tr", bufs=2, space="PSUM"))
        ps_mm = ctx.enter_context(tc.tile_pool(name="ps_mm", bufs=2, space="PSUM"))
        ps_sm = ctx.enter_context(tc.tile_pool(name="ps_sm", bufs=1, space="PSUM"))

        c_tiles = {}
        q_tiles = {}
        S = [dict() for _ in range(BPC)]  # per-batch tile state

        def issue_loads(bb):
            # q on the gpsimd queue, c on sync: parallel issue, q lands first
            q_t = qpool.tile([Lq, H], F32, name="q_sb")
            nc.gpsimd.dma_start(out=q_t, in_=q_ap[bb])
            c_t = cpool.tile([128, 4, H], F32, name="c_sb")
            nc.sync.dma_start(out=c_t, in_=c_ap[bb].rearrange("(j p) h -> p j h", p=128))
            c_tiles[bb] = c_t
            q_tiles[bb] = q_t

        # load issues first: nothing on the issue queues may delay them
        for bb in range(BPC):
            issue_loads(bb)

        # ---- constants (loaded once, reused for all batches) ----
        ident = const.tile([128, 128], BF16, name="ident")
        make_identity(nc, ident)
        # one packed DMA for the three trilinear weight vectors
        cr3 = const.tile([128, 3, 4], F32, name="cr3")
        nc.gpsimd.dma_start(
            out=cr3,
            in_=bass.AP(tensor=consts_h, offset=0, ap=[[1, 128], [H, 3], [128, 4]]),
        )
        cqw_col = cr3[:, 0, :]
        ratio_col = cr3[:, 1, :]
        cwgt_col = cr3[:, 2, :]
        ratio_bf = const.tile([128, 4], BF16, name="ratio_bf")
        nc.vector.tensor_copy(out=ratio_bf, in_=ratio_col)
        cwgt_hi = const.tile([128, 4], BF16, name="cwgt_hi")
        nc.vector.tensor_copy(out=cwgt_hi, in_=cwgt_col)
        bias_bc = const.tile([Lq, 1], F32, name="bias_bc")
        nc.gpsimd.dma_start(out=bias_bc, in_=bass.AP(tensor=bias_h, offset=0, ap=[[0, Lq], [1, 1]]))
        ones_col = const.tile([Lq, 1], BF16, name="ones_col")
        nc.vector.memset(ones_col, 1.0)
        aug_f = const.tile([1, 65], F32, name="aug_f")
        nc.vector.memset(aug_f[:, 0:64], 1.0)
        nc.vector.memset(aug_f[:, 64:65], 0.0)
        aug = const.tile([1, 65], mybir.dt.float32r, name="aug")  # rank-1 s0 add
        nc.vector.tensor_copy(out=aug, in_=aug_f)

        # persistent per-batch lhsT with the cwgt column prefilled once
        # (s0 lands in psum row 64: engine reads need 32-aligned partitions)
        lhsT_all = []
        for b in range(BPC):
            lhsT = const.tile([128, 4, 65], BF16, name=f"lhsT{b}")
            nc.gpsimd.tensor_copy(out=lhsT[:, :, 64:65], in_=cwgt_hi)
            lhsT_all.append(lhsT)

        def stage_A(b):
            """loads -> bf16 casts -> transposes -> sT matmuls -> exp"""
            c_sb = c_tiles[b]
            q_sb = q_tiles[b]
            lhsT = lhsT_all[b]

            # q path first: small load -> early PE work
            q_bf = qpool.tile([Lq, H], BF16, name="q_bf")
            nc.scalar.activation(out=q_bf, in_=q_sb, func=copy_f)

            # qs^T = (q * cqw)^T via PE transpose of q then per-partition
            # cqw scale on the PSUM->SBUF evac (into prefilled lhsT cols 0:64)
            pt_q = ps_tr.tile([128, 4, 64], BF16, name="pt_q", tag="trq", bufs=1)
            for f in range(4):
                nc.tensor.transpose(pt_q[:, f, :], q_bf[:, ts(f, 128)], ident[0:64, 0:64])
            for f in range(4):
                nc.vector.tensor_scalar_mul(
                    lhsT[:, f, 0:64], pt_q[:, f, :], cqw_col[:, f : f + 1]
                )

            # c casts before s1b in the scalar queue (s1b depends on PE work;
            # emitting it later avoids head-of-line blocking the casts)
            c_bf = cbfpool.tile([128, 4, H], BF16, name="c_bf")
            nc.scalar.activation(out=c_bf[:, 0, :], in_=c_sb[:, 0, :], func=copy_f)
            nc.vector.tensor_copy(out=c_bf[:, 1, :], in_=c_sb[:, 1, :])
            nc.gpsimd.tensor_copy(out=c_bf[:, 2, :], in_=c_sb[:, 2, :])
            nc.scalar.activation(out=c_bf[:, 3, :], in_=c_sb[:, 3, :], func=copy_f)

            # cT[f] = c^T chunk (H rows f*128.., all Lc cols), bf16
            cT = ctpool.tile([128, 4, H], BF16, name="cT")
            for j in range(4):
                pt_c = ps_tr.tile([128, 4, 128], BF16, name="pt_c", tag="tr")
                for f in range(4):
                    nc.tensor.transpose(pt_c[:, f, :], c_bf[:, j, ts(f, 128)], ident)
                nc.vector.tensor_copy(out=cT[:, :, ts(j, 128)], in_=pt_c)

            # s1 = q @ q_weight == qs^T . (qw/cqw) per column
            ps_small = ps_sm.tile([128, 5], F32, name="ps_small")
            for f in range(4):
                nc.tensor.matmul(
                    ps_small[0:64, 4:5], lhsT[:, f, 0:64], ratio_bf[:, f : f + 1],
                    start=(f == 0), stop=(f == 3),
                )
            s1b = spool.tile([Lq, 1], F32, name="s1b")
            nc.scalar.activation(
                out=s1b, in_=ps_small[0:64, 4:5], func=ident_f, bias=bias_bc, scale=1.0
            )
            S[b]["ps_small"] = ps_small

            # sT accumulation: rows 0..63 = qs@cT, row 64 = s0 = c @ cwgt
            ps_sT = ps_mm.tile([128, 512], F32, name="ps_sT", tag="big1", bufs=2)
            for f in range(4):
                nc.tensor.matmul(
                    ps_sT[0:65, :], lhsT[:, f, :], cT[:, f, :],
                    start=(f == 0), stop=False,
                )
            s0row = spool.tile([1, H], mybir.dt.float32r, name="s0row")
            nc.vector.tensor_copy(out=s0row, in_=ps_sT[64:65, :])
            nc.tensor.matmul(
                ps_sT[0:65, :], aug, s0row,
                start=False, stop=True,
            )

            # E = exp(sT + s1 + bias) in bf16; rowsum (f32) for a2
            E_sb = epool.tile([Lq, H], BF16, name="E_sb")
            rowsum = spool.tile([Lq, 1], F32, name="rowsum")
            nc.scalar.activation(
                out=E_sb, in_=ps_sT[0:64, :], func=exp_f, bias=s1b, scale=1.0,
                accum_out=rowsum,
            )
            S[b].update(c_sb=c_sb, c_bf=c_bf, q_bf=q_bf, E_sb=E_sb, rowsum=rowsum)

        def stage_B(b):
            """colsum normalizers -> E transpose -> M2 = a2^T @ c"""
            c_bf = S[b]["c_bf"]
            E_sb = S[b]["E_sb"]
            ra2 = spool.tile([Lq, 1], F32, name="ra2")
            nc.vector.reciprocal(ra2, S[b]["rowsum"])

            # column sums of E (normalizer of a1), one batched reciprocal
            ps_S = S[b]["ps_small"]
            for m in range(4):
                nc.tensor.matmul(
                    ps_S[:, m : m + 1], E_sb[:, ts(m, 128)], ones_col,
                    start=True, stop=True,
                )
            rS = spool.tile([128, 4], F32, name="rS")
            nc.vector.reciprocal(rS, ps_S[:, 0:4])

            # E in natural layout [Lc, Lq] via PE transposes (a2 normalization
            # deferred to the M2 evac scale)
            a2n = btpool.tile([128, 4, Lq], BF16, name="a2n")
            pt_a = ps_tr.tile([128, 4, 64], BF16, name="pt_a", tag="trq", bufs=1)
            for f in range(4):
                nc.tensor.transpose(pt_a[:, f, :], E_sb[:, ts(f, 128)], ident[0:64, 0:64])
            nc.scalar.activation(out=a2n, in_=pt_a, func=copy_f)
            # M2 = a2^T @ c  [Lq, H]  (b = a1 @ M2 afterwards - associativity)
            ps_M2 = ps_mm.tile([128, 512], F32, name="ps_M2", tag="big1", bufs=2)
            for jj in range(4):
                nc.tensor.matmul(
                    ps_M2[0:64, :], a2n[:, jj, :], c_bf[:, jj, :],
                    start=(jj == 0), stop=(jj == 3),
                )
            M2_bf = epool.tile([Lq, H], BF16, name="M2_bf")
            nc.scalar.activation(out=M2_bf, in_=ps_M2[0:64, :], func=copy_f, scale=ra2)
            S[b].update(rS=rS, M2_bf=M2_bf)

        def stage_C(b, ms=(0, 1, 2, 3)):
            """per i-tile: a / ca / b / cb + bf16 stores"""
            c_sb = S[b]["c_sb"]
            c_bf = S[b]["c_bf"]
            q_bf = S[b]["q_bf"]
            E_sb = S[b]["E_sb"]
            rS = S[b]["rS"]
            M2_bf = S[b]["M2_bf"]
            for m in ms:
                stage = opool.tile([128, 3, H], BF16, name="stage")
                # a = (E^T chunk @ q) * rS ; ca = c * a
                ps_a = ps_mm.tile([128, 512], F32, name="ps_a", tag="big2")
                nc.tensor.matmul(
                    ps_a, E_sb[:, ts(m, 128)], q_bf,
                    start=True, stop=True,
                )
                nc.scalar.activation(out=stage[:, 0, :], in_=ps_a, func=copy_f, scale=rS[:, m : m + 1])
                # ca from SBUF (gpsimd cannot read PSUM); alternate engines so
                # neither gpsimd (slow TT) nor vector gates the C phase
                if m % 2 == 0:
                    nc.gpsimd.tensor_mul(stage[:, 1, :], stage[:, 0, :], c_bf[:, m, :])
                else:
                    nc.vector.tensor_mul(stage[:, 1, :], stage[:, 0, :], c_bf[:, m, :])
                # b = (a1 @ M2) * rS ; cb = c * b
                ps_b = ps_mm.tile([128, 512], F32, name="ps_b", tag="big2")
                nc.tensor.matmul(
                    ps_b, E_sb[:, ts(m, 128)], M2_bf,
                    start=True, stop=True,
                )
                nc.vector.scalar_tensor_tensor(
                    out=stage[:, 2, :], in0=ps_b, scalar=rS[:, m : m + 1], in1=c_bf[:, m, :],
                    op0=MULT, op1=MULT,
                )
                # store: out = [a | c*a | c*b] (bf16; c column is host-side)
                nc.sync.dma_start(out=out_ap[b, ts(m, 128), :], in_=stage)

        # emission order: loads first (in-order issue queues), A stages early
        # (their deps are ready early) so late-stage ops never block them at
        # an engine queue head; C emitted in halves to interleave with B
        stage_A(0)
        stage_A(1)
        stage_B(0)
        stage_A(2)
        stage_B(1)
        stage_C(0, (0, 1))
        stage_A(3)
        stage_B(2)
        stage_C(0, (2, 3))
        stage_C(1, (0, 1))
        stage_B(3)
        stage_C(1, (2, 3))
        stage_C(2, (0, 1))
        stage_C(2, (2, 3))
        stage_C(3)

    nc.compile()
    return nc


def _numpy_fallback(c, q, c_mask, q_mask, c_weight, q_weight, cq_weight, bias):
    NEG_INF = -1e30
    s0 = c @ c_weight
    s1 = (q @ q_weight).transpose(0, 2, 1)
    s2 = np.einsum("bih,bjh->bij", c * cq_weight, q)
    s = s0 + s1 + s2 + bias

    def softmax(x, mask, axis):
        logits = np.where(mask, x, NEG_INF)
        m = logits.max(axis=axis, keepdims=True)
        e = np.exp(logits - m)
        return e / e.sum(axis=axis, keepdims=True)

    a1 = softmax(s, q_mask[:, None, :], 2)
    a2 = softmax(s, c_mask[:, :, None], 1)
    a = np.einsum("bij,bjh->bih", a1, q)
    bb = np.einsum("bik,bjk->bij", a1, a2)
    bb = np.einsum("bij,bjh->bih", bb, c)
    return np.concatenate([c, a, c * a, c * bb], axis=2).astype(np.float32)


def kernel(c, q, c_mask, q_mask, c_weight, q_weight, cq_weight, bias, **_):
    c = np.asarray(c, dtype=np.float32)
    q = np.asarray(q, dtype=np.float32)
    if not (np.all(c_mask) and np.all(q_mask)):
        # masks are all-ones per the problem spec; keep a correct fallback
        return _numpy_fallback(
            c, q, np.asarray(c_mask), np.asarray(q_mask),
            np.asarray(c_weight, np.float32), np.asarray(q_weight, np.float32),
            np.asarray(cq_weight, np.float32), np.asarray(bias, np.float32),
        )

    if "nc" not in _CACHE:
        _CACHE["nc"] = _build_program()
    nc = _CACHE["nc"]

    cqw = np.asarray(cq_weight, np.float32).reshape(H)
    cwgt = np.asarray(c_weight, np.float32).reshape(H)
    qwgt = np.asarray(q_weight, np.float32).reshape(H)
    ratio = np.where(
        np.abs(cqw) > 1e-30, qwgt / np.where(cqw == 0, 1.0, cqw), 0.0
    ).astype(np.float32)
    consts = np.ascontiguousarray(np.stack([cqw, ratio, cwgt]))
    bias_a = np.ascontiguousarray(np.asarray(bias, np.float32).reshape(1))

    in_maps = []
    for k in range(N_CORES):
        in_maps.append(
            {
                "c": np.ascontiguousarray(c[k * BPC : (k + 1) * BPC]),
                "q": np.ascontiguousarray(q[k * BPC : (k + 1) * BPC]),
                "consts": consts,
                "bias": bias_a,
            }
        )
    res = run_bass_kernel_spmd(nc, in_maps, core_ids=list(range(N_CORES)))
    out = np.empty((B, Lc, 4 * H), dtype=np.float32)
    out[:, :, 0:H] = c
    for k in range(N_CORES):
        out[k * BPC : (k + 1) * BPC, :, H:] = res.results[k]["out"].astype(np.float32)
    return out



# revision 5
# speedup vs baseline: 1.0746x; 1.0746x over previous
"""BiDAF attention kernel for 8 Trainium2 NeuronCores.

Data-parallel over batch (B=32 -> 4 per core). Per batch, on-chip:
  sT[j,i] = (q*cqw) @ c^T + s0[i] + (s1[j]+bias)   (bf16 matmuls, fp32 accum)
  s0 = c @ c_weight and s1b = q @ q_weight + bias are host-precomputed
  (tiny rank-1 terms); s0 enters via a K=1 f32r matmul accumulated into the
  same PSUM bank, s1b via the exp activation's per-partition bias.
  E = exp(sT)  (one exp serves both softmaxes; rowsum via accum_out)
  a1 normalization deferred: rS=1/colsum(E) scales the a/b PSUM evacuations;
  a2 normalization deferred: ra2=1/rowsum(E) folds into the M2 evacuation.
  a = a1 @ q; b = a1 @ (a2^T @ c); device stores [a, b] in bf16.
Key perf structure vs the previous version:
  - c is never cast on an engine: PE transposes c as f32r (1.5 cy/row) and
    the PSUM->SBUF evacuation casts to bf16 for the sT matmul rhs.
  - q is cast f32->bf16 in flight by the gpsimd software-DGE DMA.
  - M2 = a2^T @ c runs as an f32r matmul straight off the f32 c tile.
  - c*a and c*b are computed on the host during unshard (the device writes
    only [a, b]: 4 MiB of bf16 HBM writes per core instead of 6+).
  - identity + small consts issue ahead of the bulk loads; one batched 1 MiB
    store per batch; stages interleaved so the PE pipeline never drains
    (p-state ramp: a continuously-busy PE runs 2x faster than one with gaps).
"""

import sys

if "/opt/trn_rl_repo" not in sys.path:
    sys.path.insert(0, "/opt/trn_rl_repo")

from contextlib import ExitStack

import numpy as np

import concourse.bacc as bacc
import concourse.bass as bass
import concourse.mybir as mybir
from concourse.bass import ts
from concourse.bass_utils import run_bass_kernel_spmd
from concourse.masks import make_identity
from concourse.tile import TileContext

N_CORES = 8
B, Lc, Lq, H = 32, 512, 64, 512
BPC = B // N_CORES  # batches per core
F32 = mybir.dt.float32
F32R = mybir.dt.float32r
BF16 = mybir.dt.bfloat16

_CACHE = {}


def _build_program():
    nc = bacc.Bacc("TRN2", target_bir_lowering=False, debug=False, num_devices=N_CORES)
    c_h = nc.dram_tensor("c", [BPC, Lc, H], F32R, kind="ExternalInput")
    q_h = nc.dram_tensor("q", [BPC, Lq, H], F32, kind="ExternalInput")
    cqw_h = nc.dram_tensor("cqw", [H], F32, kind="ExternalInput")
    s0_h = nc.dram_tensor("s0", [BPC, Lc], F32R, kind="ExternalInput")
    s1b_h = nc.dram_tensor("s1b", [BPC, Lq], F32, kind="ExternalInput")
    out_h = nc.dram_tensor("out", [BPC, 4, 128, 2, H], BF16, kind="ExternalOutput")

    c_ap = c_h.ap()
    q_ap = q_h.ap()
    out_ap = out_h.ap()

    exp_f = mybir.ActivationFunctionType.Exp
    copy_f = mybir.ActivationFunctionType.Copy

    with TileContext(nc) as tc, ExitStack() as ctx:
        const = ctx.enter_context(tc.tile_pool(name="const", bufs=1))
        cpool = ctx.enter_context(tc.tile_pool(name="cpool", bufs=4))
        ctpool = ctx.enter_context(tc.tile_pool(name="ctpool", bufs=2))
        lhpool = ctx.enter_context(tc.tile_pool(name="lhpool", bufs=2))
        qpool = ctx.enter_context(tc.tile_pool(name="qpool", bufs=4))
        spool = ctx.enter_context(tc.tile_pool(name="spool", bufs=12))
        epool = ctx.enter_context(tc.tile_pool(name="epool", bufs=4))
        btpool = ctx.enter_context(tc.tile_pool(name="btpool", bufs=2))
        mpool = ctx.enter_context(tc.tile_pool(name="mpool", bufs=3))
        opool = ctx.enter_context(tc.tile_pool(name="opool", bufs=2))
        ps_tr = ctx.enter_context(tc.tile_pool(name="ps_tr", bufs=2, space="PSUM"))
        ps_trq = ctx.enter_context(tc.tile_pool(name="ps_trq", bufs=1, space="PSUM"))
        ps_mm = ctx.enter_context(tc.tile_pool(name="ps_mm", bufs=2, space="PSUM"))
        ps_ab = ctx.enter_context(tc.tile_pool(name="ps_ab", bufs=2, space="PSUM"))
        ps_sm = ctx.enter_context(tc.tile_pool(name="ps_sm", bufs=1, space="PSUM"))

        # ---- constants + loads: identity first on the gpsimd queue so the
        # first PE transposes are never gated on it; q casts f32->bf16 in
        # flight (SWDGE); c goes f32 on the sync HWDGE queue; small consts
        # issue from the scalar HWDGE queue ahead of its activation work ----
        ident = const.tile([128, 128], BF16, name="ident")
        make_identity(nc, ident)
        identf = const.tile([128, 128], F32R, name="identf")
        nc.vector.tensor_copy(out=identf, in_=ident)

        q_tiles = {}
        c_tiles = {}
        for bb in range(BPC):
            q_t = qpool.tile([Lq, H], BF16, name="q_sb")
            nc.gpsimd.dma_start(out=q_t, in_=q_ap[bb])
            q_tiles[bb] = q_t
        for bb in range(BPC):
            c_t = cpool.tile([128, 4, H], F32R, name="c_sb")
            nc.sync.dma_start(out=c_t, in_=c_ap[bb].rearrange("(j p) h -> p j h", p=128))
            c_tiles[bb] = c_t

        cqw_t = const.tile([128, 4], F32, name="cqw_t")
        nc.scalar.dma_start(
            out=cqw_t, in_=bass.AP(tensor=cqw_h, offset=0, ap=[[1, 128], [128, 4]])
        )
        s1b_t = const.tile([Lq, BPC], F32, name="s1b_t")
        nc.scalar.dma_start(
            out=s1b_t, in_=bass.AP(tensor=s1b_h, offset=0, ap=[[1, Lq], [Lq, BPC]])
        )
        s0_t = const.tile([1, BPC * Lc], F32R, name="s0_t")
        nc.scalar.dma_start(
            out=s0_t, in_=bass.AP(tensor=s0_h, offset=0, ap=[[1, 1], [1, BPC * Lc]])
        )

        ones_col = const.tile([Lq, 1], BF16, name="ones_col")
        nc.vector.memset(ones_col, 1.0)
        ones_f = const.tile([1, Lq], F32, name="ones_f")
        nc.vector.memset(ones_f, 1.0)
        onesK = const.tile([1, Lq], F32R, name="onesK")
        nc.vector.tensor_copy(out=onesK, in_=ones_f)

        S = [dict() for _ in range(BPC)]  # per-batch tile state

        def stage_A(b):
            """c transposes (f32r) -> qs^T -> sT matmuls + s0 aug -> exp"""
            c_sb = c_tiles[b]
            q_sb = q_tiles[b]

            # cT[f] = c^T chunk (h rows f*128.., all Lc cols); evac casts->bf16
            cT = ctpool.tile([128, 4, H], BF16, name="cT")
            for j in range(4):
                pt_c = ps_tr.tile([128, 4, 128], F32R, name="pt_c", tag="tr")
                for f in range(4):
                    nc.tensor.transpose(pt_c[:, f, :], c_sb[:, j, ts(f, 128)], identf)
                if j % 2 == 0:
                    nc.vector.tensor_copy(out=cT[:, :, ts(j, 128)], in_=pt_c)
                else:
                    nc.scalar.activation(
                        out=cT[:, :, ts(j, 128)], in_=pt_c, func=copy_f
                    )

            # qs^T = (q * cqw)^T via PE transpose + per-partition cqw scale
            lhsT = lhpool.tile([128, 4, Lq], BF16, name="lhsT")
            pt_q = ps_trq.tile([128, 4, Lq], BF16, name="pt_q", tag="trq")
            for f in range(4):
                nc.tensor.transpose(pt_q[:, f, :], q_sb[:, ts(f, 128)], ident[0:Lq, 0:Lq])
            for f in range(4):
                nc.vector.tensor_scalar_mul(
                    lhsT[:, f, :], pt_q[:, f, :], cqw_t[:, f : f + 1]
                )

            # sT rows 0..63 = qs @ cT; then s0 broadcast via K=1 f32r matmul
            ps_sT = ps_mm.tile([128, 512], F32, name="ps_sT", tag="big1")
            for f in range(4):
                nc.tensor.matmul(
                    ps_sT[0:Lq, :], lhsT[:, f, :], cT[:, f, :],
                    start=(f == 0), stop=False,
                )
            nc.tensor.matmul(
                ps_sT[0:Lq, :], onesK, s0_t[0:1, ts(b, Lc)],
                start=False, stop=True,
            )

            # E = exp(sT + s1b) in bf16; rowsum (f32) for a2
            E_sb = epool.tile([Lq, H], BF16, name="E_sb")
            rowsum = spool.tile([Lq, 1], F32, name="rowsum")
            nc.scalar.activation(
                out=E_sb, in_=ps_sT[0:Lq, :], func=exp_f,
                bias=s1b_t[:, b : b + 1], scale=1.0, accum_out=rowsum,
            )
            S[b].update(c_sb=c_sb, q_sb=q_sb, E_sb=E_sb, rowsum=rowsum)

        def stage_B(b):
            """normalizers -> E transpose -> M2 = a2^T @ c (f32r)"""
            c_sb = S[b]["c_sb"]
            E_sb = S[b]["E_sb"]
            ra2 = spool.tile([Lq, 1], F32, name="ra2")
            nc.vector.reciprocal(ra2, S[b]["rowsum"])

            # column sums of E (normalizer of a1), one batched reciprocal
            ps_S = ps_sm.tile([128, 4], F32, name="ps_S", tag="small")
            for m in range(4):
                nc.tensor.matmul(
                    ps_S[:, m : m + 1], E_sb[:, ts(m, 128)], ones_col,
                    start=True, stop=True,
                )
            rS = spool.tile([128, 4], F32, name="rS")
            nc.vector.reciprocal(rS, ps_S)

            # E^T chunks for M2's lhsT (f32r to match the f32 c rhs)
            pt_a = ps_trq.tile([128, 4, Lq], BF16, name="pt_a", tag="trq")
            for f in range(4):
                nc.tensor.transpose(pt_a[:, f, :], E_sb[:, ts(f, 128)], ident[0:Lq, 0:Lq])
            a2n = btpool.tile([128, 4, Lq], F32R, name="a2n")
            nc.scalar.activation(out=a2n, in_=pt_a, func=copy_f)

            # M2 = a2^T @ c  [Lq, H]; evac applies ra2, casts to bf16
            ps_M2 = ps_mm.tile([128, 512], F32, name="ps_M2", tag="big1")
            for j in range(4):
                nc.tensor.matmul(
                    ps_M2[0:Lq, :], a2n[:, j, :], c_sb[:, j, :],
                    start=(j == 0), stop=(j == 3),
                )
            M2_sb = mpool.tile([Lq, H], BF16, name="M2_sb")
            nc.scalar.activation(
                out=M2_sb, in_=ps_M2[0:Lq, :], func=copy_f, scale=ra2
            )
            S[b].update(rS=rS, M2_sb=M2_sb)

        def stage_C(b, ms=(0, 1, 2, 3)):
            """per i-tile: a = a1@q, b = a1@M2, rS-scaled bf16 evacs"""
            q_sb = S[b]["q_sb"]
            E_sb = S[b]["E_sb"]
            rS = S[b]["rS"]
            M2_sb = S[b]["M2_sb"]
            if "stage" not in S[b]:
                S[b]["stage"] = opool.tile([128, 4, 2, H], BF16, name="stage")
            stage = S[b]["stage"]
            for m in ms:
                ps_a = ps_ab.tile([128, 512], F32, name="ps_a", tag="big2")
                nc.tensor.matmul(
                    ps_a, E_sb[:, ts(m, 128)], q_sb, start=True, stop=True
                )
                nc.scalar.activation(
                    out=stage[:, m, 0, :], in_=ps_a, func=copy_f,
                    scale=rS[:, m : m + 1],
                )
                ps_b = ps_ab.tile([128, 512], F32, name="ps_b", tag="big2")
                nc.tensor.matmul(
                    ps_b, E_sb[:, ts(m, 128)], M2_sb, start=True, stop=True
                )
                nc.vector.tensor_scalar_mul(
                    stage[:, m, 1, :], ps_b, rS[:, m : m + 1]
                )
            if ms[-1] == 3:
                nc.sync.dma_start(
                    out=out_ap[b].rearrange("m p t h -> p m t h"), in_=stage
                )

        # emission order: A stages early (deps land early), C split in halves
        # to interleave with B so every engine queue always has ready work
        stage_A(0)
        stage_A(1)
        stage_B(0)
        stage_A(2)
        stage_B(1)
        stage_C(0, (0, 1))
        stage_A(3)
        stage_B(2)
        stage_C(0, (2, 3))
        stage_C(1, (0, 1))
        stage_B(3)
        stage_C(1, (2, 3))
        stage_C(2, (0, 1))
        stage_C(2, (2, 3))
        stage_C(3)

    nc.compile()
    return nc


def _numpy_fallback(c, q, c_mask, q_mask, c_weight, q_weight, cq_weight, bias):
    NEG_INF = -1e30
    s0 = c @ c_weight
    s1 = (q @ q_weight).transpose(0, 2, 1)
    s2 = np.einsum("bih,bjh->bij", c * cq_weight, q)
    s = s0 + s1 + s2 + bias

    def softmax(x, mask, axis):
        logits = np.where(mask, x, NEG_INF)
        m = logits.max(axis=axis, keepdims=True)
        e = np.exp(logits - m)
        return e / e.sum(axis=axis, keepdims=True)

    a1 = softmax(s, q_mask[:, None, :], 2)
    a2 = softmax(s, c_mask[:, :, None], 1)
    a = np.einsum("bij,bjh->bih", a1, q)
    bb = np.einsum("bij,bjk->bik", np.einsum("bik,bjk->bij", a1, a2), c)
    return np.concatenate([c, a, c * a, c * bb], axis=2).astype(np.float32)


def _make_in_maps(c, q, cq_weight, c_weight, q_weight, bias):
    cqw = np.ascontiguousarray(np.asarray(cq_weight, np.float32).reshape(H))
    cwgt = np.asarray(c_weight, np.float32).reshape(H)
    qwgt = np.asarray(q_weight, np.float32).reshape(H)
    b0 = float(np.asarray(bias, np.float32).reshape(1)[0])
    s0 = (c.reshape(-1, H) @ cwgt).reshape(B, Lc).astype(np.float32)
    s1b = ((q.reshape(-1, H) @ qwgt).reshape(B, Lq) + b0).astype(np.float32)
    in_maps = []
    for k in range(N_CORES):
        sl = slice(k * BPC, (k + 1) * BPC)
        in_maps.append(
            {
                "c": np.ascontiguousarray(c[sl]),
                "q": np.ascontiguousarray(q[sl]),
                "cqw": cqw,
                "s0": np.ascontiguousarray(s0[sl]),
                "s1b": np.ascontiguousarray(s1b[sl]),
            }
        )
    return in_maps


def _assemble(c, results):
    out = np.empty((B, Lc, 4 * H), dtype=np.float32)
    out[:, :, 0:H] = c
    for k in range(N_CORES):
        sl = slice(k * BPC, (k + 1) * BPC)
        ab = results[k]["out"].reshape(BPC, Lc, 2, H).astype(np.float32)
        a = ab[:, :, 0, :]
        bb = ab[:, :, 1, :]
        ck = c[sl]
        out[sl, :, H : 2 * H] = a
        np.multiply(ck, a, out=out[sl, :, 2 * H : 3 * H])
        np.multiply(ck, bb, out=out[sl, :, 3 * H : 4 * H])
    return out


def kernel(c, q, c_mask, q_mask, c_weight, q_weight, cq_weight, bias, **_):
    c = np.asarray(c, dtype=np.float32)
    q = np.asarray(q, dtype=np.float32)
    if not (np.all(c_mask) and np.all(q_mask)):
        # masks are all-ones per the problem spec; keep a correct fallback
        return _numpy_fallback(
            c, q, np.asarray(c_mask), np.asarray(q_mask),
            np.asarray(c_weight, np.float32), np.asarray(q_weight, np.float32),
            np.asarray(cq_weight, np.float32), np.asarray(bias, np.float32),
        )

    if "nc" not in _CACHE:
        _CACHE["nc"] = _build_program()
    nc = _CACHE["nc"]

    in_maps = _make_in_maps(c, q, cq_weight, c_weight, q_weight, bias)
    res = run_bass_kernel_spmd(nc, in_maps, core_ids=list(range(N_CORES)))
    return _assemble(c, res.results)


# revision 7
# speedup vs baseline: 1.1895x; 1.1069x over previous
"""BiDAF attention kernel for 8 Trainium2 NeuronCores.

Data-parallel over batch (B=32 -> 4 per core). Per batch, on-chip:
  sT[j,i] = (q*cqw) @ c^T + s0[i] + (s1[j]+bias)   (bf16 matmuls, fp32 accum)
  s0 = c @ c_weight and s1b = q @ q_weight + bias are host-precomputed
  (tiny rank-1 terms); s0 enters via a K=1 f32r matmul accumulated into the
  same PSUM bank, s1b via the exp activation's per-partition bias.
  E = exp(sT)  (one exp serves both softmaxes; rowsum via accum_out)
  a1 normalization deferred: rS=1/colsum(E) scales the a/b PSUM evacuations;
  a2 normalization deferred: ra2=1/rowsum(E) folds into the M2 evacuation.
  a = a1 @ q; b = a1 @ (a2^T @ c); device stores [a, b] in bf16.
Key perf structure vs the previous version:
  - c is never cast on an engine: PE transposes c as f32r (1.5 cy/row) and
    the PSUM->SBUF evacuation casts to bf16 for the sT matmul rhs.
  - q is cast f32->bf16 in flight by the gpsimd software-DGE DMA.
  - M2 = a2^T @ c runs as an f32r matmul straight off the f32 c tile.
  - c*a and c*b are computed on the host during unshard (the device writes
    only [a, b]: 4 MiB of bf16 HBM writes per core instead of 6+).
  - identity + small consts issue ahead of the bulk loads; one batched 1 MiB
    store per batch; stages interleaved so the PE pipeline never drains
    (p-state ramp: a continuously-busy PE runs 2x faster than one with gaps).
"""

import sys

if "/opt/trn_rl_repo" not in sys.path:
    sys.path.insert(0, "/opt/trn_rl_repo")

from contextlib import ExitStack

import numpy as np

import concourse.bacc as bacc
import concourse.bass as bass
import concourse.mybir as mybir
from concourse.bass import ts
from concourse.bass_utils import run_bass_kernel_spmd
from concourse.masks import make_identity
from concourse.tile import TileContext

N_CORES = 8
B, Lc, Lq, H = 32, 512, 64, 512
BPC = B // N_CORES  # batches per core
F32 = mybir.dt.float32
F32R = mybir.dt.float32r
BF16 = mybir.dt.bfloat16

_CACHE = {}


def _build_program():
    nc = bacc.Bacc("TRN2", target_bir_lowering=False, debug=False, num_devices=N_CORES)
    c_h = nc.dram_tensor("c", [BPC, Lc, H], F32R, kind="ExternalInput")
    q_h = nc.dram_tensor("q", [BPC, Lq, H], F32, kind="ExternalInput")
    cqw_h = nc.dram_tensor("cqw", [H], F32, kind="ExternalInput")
    s0_h = nc.dram_tensor("s0", [BPC, Lc], F32R, kind="ExternalInput")
    s1b_h = nc.dram_tensor("s1b", [BPC, Lq], F32, kind="ExternalInput")
    out_h = nc.dram_tensor("out", [BPC, 4, 128, 2, H], BF16, kind="ExternalOutput")

    c_ap = c_h.ap()
    q_ap = q_h.ap()
    out_ap = out_h.ap()

    exp_f = mybir.ActivationFunctionType.Exp
    copy_f = mybir.ActivationFunctionType.Copy

    with TileContext(nc) as tc, ExitStack() as ctx:
        const = ctx.enter_context(tc.tile_pool(name="const", bufs=1))
        cpool = ctx.enter_context(tc.tile_pool(name="cpool", bufs=4))
        ctpool = ctx.enter_context(tc.tile_pool(name="ctpool", bufs=2))
        lhpool = ctx.enter_context(tc.tile_pool(name="lhpool", bufs=2))
        qpool = ctx.enter_context(tc.tile_pool(name="qpool", bufs=4))
        spool = ctx.enter_context(tc.tile_pool(name="spool", bufs=12))
        epool = ctx.enter_context(tc.tile_pool(name="epool", bufs=4))
        btpool = ctx.enter_context(tc.tile_pool(name="btpool", bufs=2))
        mpool = ctx.enter_context(tc.tile_pool(name="mpool", bufs=3))
        opool = ctx.enter_context(tc.tile_pool(name="opool", bufs=2))
        ps_tr = ctx.enter_context(tc.tile_pool(name="ps_tr", bufs=2, space="PSUM"))
        ps_trq = ctx.enter_context(tc.tile_pool(name="ps_trq", bufs=1, space="PSUM"))
        ps_mm = ctx.enter_context(tc.tile_pool(name="ps_mm", bufs=2, space="PSUM"))
        ps_ab = ctx.enter_context(tc.tile_pool(name="ps_ab", bufs=2, space="PSUM"))
        ps_sm = ctx.enter_context(tc.tile_pool(name="ps_sm", bufs=1, space="PSUM"))

        # ---- constants + loads: identity first on the gpsimd queue so the
        # first PE transposes are never gated on it; q casts f32->bf16 in
        # flight (SWDGE); c goes f32 on the sync HWDGE queue; small consts
        # issue from the scalar HWDGE queue ahead of its activation work ----
        ident = const.tile([128, 128], BF16, name="ident")
        make_identity(nc, ident)
        identf = const.tile([128, 128], F32R, name="identf")
        nc.vector.tensor_copy(out=identf, in_=ident)

        q_tiles = {}
        c_tiles = {}
        for bb in range(BPC):
            q_t = qpool.tile([Lq, H], BF16, name="q_sb")
            nc.gpsimd.dma_start(out=q_t, in_=q_ap[bb])
            q_tiles[bb] = q_t
        # serialize the c loads: concurrent DMAs split HBM bandwidth evenly,
        # which would delay c0 (and the first PE work) to when ALL loads
        # finish. A 16-byte dummy DMA reading the previous c tile gates each
        # issue on the in-order sync queue until that transfer completes.
        dummy = const.tile([1, 4], F32R, name="dummy")
        for bb in range(BPC):
            c_t = cpool.tile([128, 4, H], F32R, name="c_sb")
            nc.sync.dma_start(out=c_t, in_=c_ap[bb].rearrange("(j p) h -> p j h", p=128))
            c_tiles[bb] = c_t
            if bb < BPC - 1:
                nc.sync.dma_start(out=dummy, in_=c_t[0:1, 0, 0:4])

        cqw_t = const.tile([128, 4], F32, name="cqw_t")
        nc.scalar.dma_start(
            out=cqw_t, in_=bass.AP(tensor=cqw_h, offset=0, ap=[[1, 128], [128, 4]])
        )
        s1b_t = const.tile([Lq, BPC], F32, name="s1b_t")
        nc.scalar.dma_start(
            out=s1b_t, in_=bass.AP(tensor=s1b_h, offset=0, ap=[[1, Lq], [Lq, BPC]])
        )
        s0_t = const.tile([1, BPC * Lc], F32R, name="s0_t")
        nc.scalar.dma_start(
            out=s0_t, in_=bass.AP(tensor=s0_h, offset=0, ap=[[1, 1], [1, BPC * Lc]])
        )

        ones_col = const.tile([Lq, 1], BF16, name="ones_col")
        nc.vector.memset(ones_col, 1.0)
        ones_f = const.tile([1, Lq], F32, name="ones_f")
        nc.vector.memset(ones_f, 1.0)
        onesK = const.tile([1, Lq], F32R, name="onesK")
        nc.vector.tensor_copy(out=onesK, in_=ones_f)

        S = [dict() for _ in range(BPC)]  # per-batch tile state

        def stage_A(b):
            """c transposes (f32r) -> qs^T -> sT matmuls + s0 aug -> exp"""
            c_sb = c_tiles[b]
            q_sb = q_tiles[b]

            # cT[f] = c^T chunk (h rows f*128.., all Lc cols); evac casts->bf16
            cT = ctpool.tile([128, 4, H], BF16, name="cT")
            for j in range(4):
                pt_c = ps_tr.tile([128, 4, 128], F32R, name="pt_c", tag="tr")
                for f in range(4):
                    nc.tensor.transpose(pt_c[:, f, :], c_sb[:, j, ts(f, 128)], identf)
                if j % 2 == 0:
                    nc.vector.tensor_copy(out=cT[:, :, ts(j, 128)], in_=pt_c)
                else:
                    nc.scalar.activation(
                        out=cT[:, :, ts(j, 128)], in_=pt_c, func=copy_f
                    )

            # qs^T = (q * cqw)^T via PE transpose + per-partition cqw scale
            lhsT = lhpool.tile([128, 4, Lq], BF16, name="lhsT")
            pt_q = ps_trq.tile([128, 4, Lq], BF16, name="pt_q", tag="trq")
            for f in range(4):
                nc.tensor.transpose(pt_q[:, f, :], q_sb[:, ts(f, 128)], ident[0:Lq, 0:Lq])
            for f in range(4):
                nc.vector.tensor_scalar_mul(
                    lhsT[:, f, :], pt_q[:, f, :], cqw_t[:, f : f + 1]
                )

            # sT rows 0..63 = qs @ cT; then s0 broadcast via K=1 f32r matmul
            ps_sT = ps_mm.tile([128, 512], F32, name="ps_sT", tag="big1")
            for f in range(4):
                nc.tensor.matmul(
                    ps_sT[0:Lq, :], lhsT[:, f, :], cT[:, f, :],
                    start=(f == 0), stop=False,
                )
            nc.tensor.matmul(
                ps_sT[0:Lq, :], onesK, s0_t[0:1, ts(b, Lc)],
                start=False, stop=True,
            )

            # E = exp(sT + s1b) in bf16; rowsum (f32) for a2
            E_sb = epool.tile([Lq, H], BF16, name="E_sb")
            rowsum = spool.tile([Lq, 1], F32, name="rowsum")
            nc.scalar.activation(
                out=E_sb, in_=ps_sT[0:Lq, :], func=exp_f,
                bias=s1b_t[:, b : b + 1], scale=1.0, accum_out=rowsum,
            )
            S[b].update(c_sb=c_sb, q_sb=q_sb, E_sb=E_sb, rowsum=rowsum)

        def stage_B(b):
            """normalizers -> E transpose -> M2 = a2^T @ c (f32r)"""
            c_sb = S[b]["c_sb"]
            E_sb = S[b]["E_sb"]
            ra2 = spool.tile([Lq, 1], F32, name="ra2")
            nc.vector.reciprocal(ra2, S[b]["rowsum"])

            # column sums of E (normalizer of a1), one batched reciprocal
            ps_S = ps_sm.tile([128, 4], F32, name="ps_S", tag="small")
            for m in range(4):
                nc.tensor.matmul(
                    ps_S[:, m : m + 1], E_sb[:, ts(m, 128)], ones_col,
                    start=True, stop=True,
                )
            rS = spool.tile([128, 4], F32, name="rS")
            nc.vector.reciprocal(rS, ps_S)

            # E^T chunks for M2's lhsT (f32r to match the f32 c rhs)
            pt_a = ps_trq.tile([128, 4, Lq], BF16, name="pt_a", tag="trq")
            for f in range(4):
                nc.tensor.transpose(pt_a[:, f, :], E_sb[:, ts(f, 128)], ident[0:Lq, 0:Lq])
            a2n = btpool.tile([128, 4, Lq], F32R, name="a2n")
            nc.scalar.activation(out=a2n, in_=pt_a, func=copy_f)

            # M2 = a2^T @ c  [Lq, H]; evac applies ra2, casts to bf16
            ps_M2 = ps_mm.tile([128, 512], F32, name="ps_M2", tag="big1")
            for j in range(4):
                nc.tensor.matmul(
                    ps_M2[0:Lq, :], a2n[:, j, :], c_sb[:, j, :],
                    start=(j == 0), stop=(j == 3),
                )
            M2_sb = mpool.tile([Lq, H], BF16, name="M2_sb")
            nc.scalar.activation(
                out=M2_sb, in_=ps_M2[0:Lq, :], func=copy_f, scale=ra2
            )
            S[b].update(rS=rS, M2_sb=M2_sb)

        def stage_C(b, ms=(0, 1, 2, 3)):
            """per i-tile: a = a1@q, b = a1@M2, rS-scaled bf16 evacs"""
            q_sb = S[b]["q_sb"]
            E_sb = S[b]["E_sb"]
            rS = S[b]["rS"]
            M2_sb = S[b]["M2_sb"]
            if "stage" not in S[b]:
                S[b]["stage"] = opool.tile([128, 4, 2, H], BF16, name="stage")
            stage = S[b]["stage"]
            for m in ms:
                ps_a = ps_ab.tile([128, 512], F32, name="ps_a", tag="big2")
                nc.tensor.matmul(
                    ps_a, E_sb[:, ts(m, 128)], q_sb, start=True, stop=True
                )
                nc.scalar.activation(
                    out=stage[:, m, 0, :], in_=ps_a, func=copy_f,
                    scale=rS[:, m : m + 1],
                )
                ps_b = ps_ab.tile([128, 512], F32, name="ps_b", tag="big2")
                nc.tensor.matmul(
                    ps_b, E_sb[:, ts(m, 128)], M2_sb, start=True, stop=True
                )
                nc.vector.tensor_scalar_mul(
                    stage[:, m, 1, :], ps_b, rS[:, m : m + 1]
                )
            # half-batch stores start HBM writes as soon as two i-tiles are
            # done instead of waiting for the full batch
            o_view = out_ap[b].rearrange("m p t h -> p m t h")
            if ms[-1] == 1:
                nc.sync.dma_start(out=o_view[:, 0:2], in_=stage[:, 0:2])
            elif ms[-1] == 3:
                if ms[0] == 0:
                    nc.sync.dma_start(out=o_view[:, 0:2], in_=stage[:, 0:2])
                nc.sync.dma_start(out=o_view[:, 2:4], in_=stage[:, 2:4])

        # emission order: A stages early (deps land early), C split in halves
        # to interleave with B so every engine queue always has ready work
        stage_A(0)
        stage_A(1)
        stage_B(0)
        stage_A(2)
        stage_B(1)
        stage_C(0, (0, 1))
        stage_A(3)
        stage_B(2)
        stage_C(0, (2, 3))
        stage_C(1, (0, 1))
        stage_B(3)
        stage_C(1, (2, 3))
        stage_C(2, (0, 1))
        stage_C(2, (2, 3))
        stage_C(3)

    nc.compile()
    return nc


def _numpy_fallback(c, q, c_mask, q_mask, c_weight, q_weight, cq_weight, bias):
    NEG_INF = -1e30
    s0 = c @ c_weight
    s1 = (q @ q_weight).transpose(0, 2, 1)
    s2 = np.einsum("bih,bjh->bij", c * cq_weight, q)
    s = s0 + s1 + s2 + bias

    def softmax(x, mask, axis):
        logits = np.where(mask, x, NEG_INF)
        m = logits.max(axis=axis, keepdims=True)
        e = np.exp(logits - m)
        return e / e.sum(axis=axis, keepdims=True)

    a1 = softmax(s, q_mask[:, None, :], 2)
    a2 = softmax(s, c_mask[:, :, None], 1)
    a = np.einsum("bij,bjh->bih", a1, q)
    bb = np.einsum("bij,bjk->bik", np.einsum("bik,bjk->bij", a1, a2), c)
    return np.concatenate([c, a, c * a, c * bb], axis=2).astype(np.float32)


def _make_in_maps(c, q, cq_weight, c_weight, q_weight, bias):
    cqw = np.ascontiguousarray(np.asarray(cq_weight, np.float32).reshape(H))
    cwgt = np.asarray(c_weight, np.float32).reshape(H)
    qwgt = np.asarray(q_weight, np.float32).reshape(H)
    b0 = float(np.asarray(bias, np.float32).reshape(1)[0])
    s0 = (c.reshape(-1, H) @ cwgt).reshape(B, Lc).astype(np.float32)
    s1b = ((q.reshape(-1, H) @ qwgt).reshape(B, Lq) + b0).astype(np.float32)
    in_maps = []
    for k in range(N_CORES):
        sl = slice(k * BPC, (k + 1) * BPC)
        in_maps.append(
            {
                "c": np.ascontiguousarray(c[sl]),
                "q": np.ascontiguousarray(q[sl]),
                "cqw": cqw,
                "s0": np.ascontiguousarray(s0[sl]),
                "s1b": np.ascontiguousarray(s1b[sl]),
            }
        )
    return in_maps


def _assemble(c, results):
    out = np.empty((B, Lc, 4 * H), dtype=np.float32)
    out[:, :, 0:H] = c
    for k in range(N_CORES):
        sl = slice(k * BPC, (k + 1) * BPC)
        ab = results[k]["out"].reshape(BPC, Lc, 2, H).astype(np.float32)
        a = ab[:, :, 0, :]
        bb = ab[:, :, 1, :]
        ck = c[sl]
        out[sl, :, H : 2 * H] = a
        np.multiply(ck, a, out=out[sl, :, 2 * H : 3 * H])
        np.multiply(ck, bb, out=out[sl, :, 3 * H : 4 * H])
    return out


def kernel(c, q, c_mask, q_mask, c_weight, q_weight, cq_weight, bias, **_):
    c = np.asarray(c, dtype=np.float32)
    q = np.asarray(q, dtype=np.float32)
    if not (np.all(c_mask) and np.all(q_mask)):
        # masks are all-ones per the problem spec; keep a correct fallback
        return _numpy_fallback(
            c, q, np.asarray(c_mask), np.asarray(q_mask),
            np.asarray(c_weight, np.float32), np.asarray(q_weight, np.float32),
            np.asarray(cq_weight, np.float32), np.asarray(bias, np.float32),
        )

    if "nc" not in _CACHE:
        _CACHE["nc"] = _build_program()
    nc = _CACHE["nc"]

    in_maps = _make_in_maps(c, q, cq_weight, c_weight, q_weight, bias)
    res = run_bass_kernel_spmd(nc, in_maps, core_ids=list(range(N_CORES)))
    return _assemble(c, res.results)
